# revision 1
# baseline (speedup 1.0000x reference)
"""Trainium2 distributed kernel for AnatomicalConsistencyLoss.

Sharding: 8 cores = (batch b in {0,1}) x (depth quarter q in {0..3}).
Each core processes 40 output D-planes (full H,W) of one batch element.

Device layout: each core's shard is pre-blocked on the host into 128
partition-blocks, each a 22x22x22 (d,h,w) bf16 subvolume = a 20^3 output
block plus a 1-voxel halo on every side (zeros at global volume edges).
All three separable 3-tap Sobel passes then become contiguous 1-D
shifted tensor_tensor ops on VectorE with even element offsets (DVE 2x
bf16 mode); ScalarE handles squares/ln/exp and the fused per-partition
accumulations; GpSimd is deliberately idle (its SBUF port locks out
VectorE).  The loss is decomposed as
  mag = sum(s_p) + sum(s_t) - 2*sum(sqrt(s_p*s_t)),
  cos = sum(dot * exp(-0.5*ln(s_p*s_t))),
so sqrt/rsqrt share one Ln and all reductions ride ScalarE accum_out.

Per-core output: [128, 16] fp32 partial sums (4 d-slabs x {sum s_p,
sum s_t, sum sqrt(q), sum cos}) which the host reduces to the scalar.
"""

import sys

import numpy as np

sys.path.insert(0, "/opt/trn_rl_repo")

import ml_dtypes

N_CORES = 8
BC = 20          # block core size
BB = 22          # block size with halo
FB = BB * BB * BB
NVOX = 2 * 160 * 160 * 160
WEIGHT = 0.2
EPS_MAG = 1e-8

_cache = {}


def _build():
    import concourse.bacc as bacc
    import concourse.bass as bass
    import concourse.tile as tile
    from concourse import mybir

    f32 = mybir.dt.float32
    bf16 = mybir.dt.bfloat16
    AF = mybir.ActivationFunctionType
    ALU = mybir.AluOpType

    nc = bacc.Bacc(
        "TRN2",
        target_bir_lowering=False,
        debug=False,
        enable_asserts=False,
        num_devices=N_CORES,
    )
    xp_d = nc.dram_tensor("pred", [128, FB], bf16, kind="ExternalInput")
    xt_d = nc.dram_tensor("targ", [128, FB], bf16, kind="ExternalInput")
    out_d = nc.dram_tensor("out", [128, 16], f32, kind="ExternalOutput")

    NSLAB = 4
    SD = BC // NSLAB       # output planes per slab (5)
    SDH = SD + 2           # input planes per slab (7)
    PL = BB * BB           # 484
    FS = SDH * PL          # 3388: slab flat size, [7, 22, 22] layout
    HS = BB                # h shift = 22
    DS = PL                # d shift = 484
    W1 = FS - 2            # 3386: W-pass extent
    H1 = W1 - HS           # 3364
    H2 = H1 - HS           # 3342
    D1 = H2 - DS           # 2858
    D2 = D1 - DS           # 2374: field valid extent
    FF = SD * PL           # 2420: field tile size
    NVV = 2000             # valid voxels per slab per block

    with tile.TileContext(nc) as tc:
        with tc.tile_pool(name="pers", bufs=1) as pers, \
             tc.tile_pool(name="work", bufs=1) as work, \
             tc.tile_pool(name="fld", bufs=2) as fld:
            accs = pers.tile([128, 4 * NSLAB], f32, tag="accs")

            xs = {}
            cuts = [0, 7 * PL, 12 * PL, 17 * PL, FB]
            for name, dram in (("p", xp_d), ("t", xt_d)):
                x = pers.tile([128, FB], bf16, tag=f"x_{name}")
                for ci in range(4):
                    nc.sync.dma_start(out=x[:, cuts[ci]:cuts[ci + 1]],
                                      in_=dram[:, cuts[ci]:cuts[ci + 1]])
                xs[name] = x

            def conv_fields(name, s):
                """Separable Sobel, one input, one d-slab, all on VectorE.

                Flat 1-D contiguous bf16 ops with even element offsets so
                every op hits a DVE fast mode (GpSimd is kept idle: its
                SBUF access locks out the DVE port pair).  Tensors keep
                junk lanes at block h/w edges ([*,22,22] blocks).
                Returns gx,gy,gz [128, FF] tiles, valid flat [0:D2].
                """
                xf = xs[name][:, SD * PL * s:SD * PL * s + FS]
                t = work.tile([128, FS], bf16, tag="t")
                nc.vector.tensor_add(t[:, 0:W1], xf[:, 0:W1], xf[:, 2:FS])
                u2 = work.tile([128, FS], bf16, tag="u2")
                nc.scalar.activation(u2[:, 0:W1], xf[:, 1:FS - 1], AF.Identity,
                                     scale=2.0)
                sw = work.tile([128, FS], bf16, tag="sw")
                nc.vector.tensor_add(sw[:, 0:W1], t[:, 0:W1], u2[:, 0:W1])
                dw = work.tile([128, FS], bf16, tag="t")
                nc.vector.tensor_sub(dw[:, 0:W1], xf[:, 2:FS], xf[:, 0:W1])

                uh1 = work.tile([128, FS], bf16, tag="uh1")
                nc.vector.tensor_add(uh1[:, 0:H1], sw[:, 0:H1], sw[:, HS:W1])
                shsw = work.tile([128, FS], bf16, tag="shsw")
                dhsw = work.tile([128, FS], bf16, tag="dhsw")
                nc.vector.tensor_add(shsw[:, 0:H2], uh1[:, 0:H2], uh1[:, HS:H1])
                nc.vector.tensor_sub(dhsw[:, 0:H2], uh1[:, HS:H1], uh1[:, 0:H2])
                uh2 = work.tile([128, FS], bf16, tag="uh1")
                nc.vector.tensor_add(uh2[:, 0:H1], dw[:, 0:H1], dw[:, HS:W1])
                shdw = work.tile([128, FS], bf16, tag="sw")
                nc.vector.tensor_add(shdw[:, 0:H2], uh2[:, 0:H2], uh2[:, HS:H1])

                ud1 = work.tile([128, D1], bf16, tag="ud1")
                nc.vector.tensor_add(ud1[:], shdw[:, 0:D1], shdw[:, DS:H2])
                gx = fld.tile([128, FF], bf16, tag=f"gx_{name}")
                nc.vector.tensor_add(gx[:, 0:D2], ud1[:, 0:D2], ud1[:, DS:D1])
                ud2 = work.tile([128, D1], bf16, tag="ud1")
                nc.vector.tensor_add(ud2[:], dhsw[:, 0:D1], dhsw[:, DS:H2])
                gy = fld.tile([128, FF], bf16, tag=f"gy_{name}")
                nc.vector.tensor_add(gy[:, 0:D2], ud2[:, 0:D2], ud2[:, DS:D1])
                gz = fld.tile([128, FF], bf16, tag=f"gz_{name}")
                nc.vector.tensor_sub(gz[:, 0:D2], shsw[:, 2 * DS:2 * DS + D2],
                                     shsw[:, 0:D2])
                return gx, gy, gz

            def valid(tt):
                """[5,20,20] strided view (excludes h/w junk lanes)."""
                return tt[:].rearrange("p (d h w) -> p d h w",
                                       d=SD, h=BB)[:, :, 0:BC, 0:BC]

            for s in range(NSLAB):
                P = conv_fields("p", s)
                T = conv_fields("t", s)

                # |grad|^2: squares on ScalarE, adds on VectorE (flat 2x)
                def sumsq(name, G):
                    sqs = []
                    for i, g in enumerate(G):
                        sq = work.tile([128, D2], bf16, tag=f"sq{i}")
                        nc.scalar.activation(sq[:], g[:, 0:D2], AF.Square)
                        sqs.append(sq)
                    s01 = work.tile([128, D2], bf16, tag="s01")
                    nc.vector.tensor_add(s01[:], sqs[0][:], sqs[1][:])
                    ss = work.tile([128, FF], bf16, tag=f"s_{name}")
                    nc.vector.tensor_add(ss[:, 0:D2], s01[:], sqs[2][:])
                    return ss

                s_p = sumsq("p", P)
                s_t = sumsq("t", T)
                junk = work.tile([128, SD, BC, BC], bf16, tag="junko")
                nc.scalar.activation(junk[:], valid(s_p), AF.Identity,
                                     accum_out=accs[:, s:s + 1])
                nc.scalar.activation(junk[:], valid(s_t), AF.Identity,
                                     accum_out=accs[:, NSLAB + s:NSLAB + s + 1])

                # dot product (all VectorE, flat even)
                m1 = work.tile([128, D2], bf16, tag="sq0")
                m2 = work.tile([128, D2], bf16, tag="sq1")
                m3 = work.tile([128, D2], bf16, tag="sq2")
                nc.vector.tensor_mul(m1[:], P[0][:, 0:D2], T[0][:, 0:D2])
                nc.vector.tensor_mul(m2[:], P[1][:, 0:D2], T[1][:, 0:D2])
                nc.vector.tensor_mul(m3[:], P[2][:, 0:D2], T[2][:, 0:D2])
                m12 = work.tile([128, D2], bf16, tag="s01")
                nc.vector.tensor_add(m12[:], m1[:], m2[:])
                dot = work.tile([128, D2], bf16, tag="sq0")
                nc.vector.tensor_add(dot[:], m12[:], m3[:])

                # q = s_p*s_t ; ln(q) shared by sqrt(q) (mag) and rsqrt (cos)
                q = work.tile([128, FF], bf16, tag="q")
                nc.vector.tensor_mul(q[:, 0:D2], s_p[:, 0:D2], s_t[:, 0:D2])
                lnq = work.tile([128, FF], f32, tag="lnq")
                nc.scalar.activation(lnq[:, 0:D2], q[:, 0:D2], AF.Ln)
                # sum of sqrt(q) over valid voxels (mag cross-term)
                nc.scalar.activation(junk[:], valid(lnq), AF.Exp,
                                     scale=0.5,
                                     accum_out=accs[:, 2 * NSLAB + s:
                                                    2 * NSLAB + s + 1])
                r = work.tile([128, D2], bf16, tag="sq1")
                nc.scalar.activation(r[:], lnq[:, 0:D2], AF.Exp, scale=-0.5)
                c = work.tile([128, FF], bf16, tag="q")
                nc.vector.tensor_mul(c[:, 0:D2], dot[:], r[:])
                nc.scalar.activation(junk[:], valid(c), AF.Identity,
                                     accum_out=accs[:, 3 * NSLAB + s:
                                                    3 * NSLAB + s + 1])

            nc.sync.dma_start(out=out_d[:], in_=accs[:])

    nc.compile()
    return nc


def _shard_inputs(pred, target):
    """Blocked bf16 shards for the 8 cores."""
    bf = ml_dtypes.bfloat16
    starts = np.arange(0, 160, BC)  # 8 block starts per axis

    in_maps = []
    blocked = {}
    for name, x in (("pred", pred), ("targ", target)):
        per_b = []
        for b in range(2):
            gp = np.zeros((162, 162, 162), np.float32)
            gp[1:161, 1:161, 1:161] = x[b, 0]
            sw = np.lib.stride_tricks.sliding_window_view(gp, (BB, BB, BB))
            per_b.append(sw)
        blocked[name] = per_b

    for core in range(N_CORES):
        b, q = divmod(core, 4)
        m = {}
        for name in ("pred", "targ"):
            sw = blocked[name][b]
            blk = sw[np.ix_([40 * q, 40 * q + BC], starts, starts)]
            m[name] = np.ascontiguousarray(
                blk.reshape(128, FB).astype(bf))
        in_maps.append(m)
    return in_maps


def run(pred, target, trace=False):
    from concourse.bass_utils import run_bass_kernel_spmd

    pred = np.asarray(pred, dtype=np.float32)
    target = np.asarray(target, dtype=np.float32)
    assert pred.shape == (2, 1, 160, 160, 160)

    if "nc" not in _cache:
        _cache["nc"] = _build()
    nc = _cache["nc"]

    in_maps = _shard_inputs(pred, target)
    res = None
    for attempt in range(3):
        try:
            res = run_bass_kernel_spmd(
                nc, in_maps, core_ids=list(range(N_CORES)), trace=trace)
            break
        except Exception:
            if attempt == 2:
                raise
            import time as _time
            _time.sleep(5)

    sp_sum = 0.0
    st_sum = 0.0
    sq_sum = 0.0
    cos_sum = 0.0
    for core_out in res.results:
        o = np.asarray(core_out["out"], np.float64)
        sp_sum += o[:, 0:4].sum()
        st_sum += o[:, 4:8].sum()
        sq_sum += o[:, 8:12].sum()
        cos_sum += o[:, 12:16].sum()

    mag_sum = sp_sum + st_sum - 2.0 * sq_sum
    loss = WEIGHT * (mag_sum / NVOX + 1.0 - cos_sum / NVOX)
    return np.float32(loss), res.exec_time_ns


def kernel(pred, target):
    loss, _ = run(pred, target, trace=False)
    return loss



# revision 13
# speedup vs baseline: 1.1263x; 1.1263x over previous
"""Trainium2 distributed kernel for AnatomicalConsistencyLoss (v2).

Sharding: 8 cores = (batch b in {0,1}) x (depth quarter q in {0..3});
each core owns 40 output D-planes (full H,W) of one batch element.

Per-core layout: partitions p = hb*42 + dl (3 h-blocks x 42 d-planes
incl 1-plane halo), free axis = (h_local 56 incl halo, w 164 padded)
bf16.  The Sobel separable conv is split across engines:
  - W passes (stride-1 axis): VectorE shifted adds at DVE 2x bf16 mode,
    with the odd-offset center tap (2*x<<1) on ScalarE.
  - H passes: VectorE shifted adds at even 164-elem offsets (2x mode).
  - D passes: TensorE matmuls with banded [126,126] conv matrices
    (S=[1,2,1], D=[-1,0,1] per h-block, zero columns at d-halo
    outputs), streaming 3-h-row chunks into PSUM fp32.
Squares run on ScalarE straight out of PSUM (fused valid-region
compaction + accum_out partial sums); dot products are DVE muls from
PSUM; sqrt on ScalarE (+accum for the mag cross term); 1/sqrt via the
custom-DVE fast reciprocal; the cosine sum via tensor_tensor_reduce.

Per-core output: [128, 160] fp32 accum slots (8 per h-band x 18 bands:
3x sum gp^2, 3x sum gt^2, sum sqrt(q), sum dot/sqrt(q)); host reduces.
"""

import sys

import numpy as np

sys.path.insert(0, "/opt/trn_rl_repo")

import ml_dtypes

N_CORES = 8
DL = 42            # d planes incl halo
HB = 3             # h blocks
HL = 56            # h_local rows incl halo
WR = 164           # padded w row (4B-aligned rows)
NP_ = 126          # used partitions
FREE = HL * WR     # 9184
NBAND = 18         # 54 valid h rows / 3
BH = 3             # h rows per band
BF = BH * WR       # 492 cols per PSUM tile (<= 512 fp32 bank)
CF = BH * 160      # 480 compact cols
NVOX = 2 * 160 * 160 * 160
WEIGHT = 0.2

_cache = {}


def _build_M():
    MS = np.zeros((128, 252), np.float32)
    for hb in range(HB):
        for do in range(40):
            j = hb * DL + do
            MS[hb * DL + do, j] += 1.0
            MS[hb * DL + do + 1, j] += 2.0
            MS[hb * DL + do + 2, j] += 1.0
            MS[hb * DL + do, 126 + j] += -1.0
            MS[hb * DL + do + 2, 126 + j] += 1.0
    return MS


def _build():
    import concourse.bacc as bacc
    import concourse.tile as tile
    from concourse import mybir

    f32 = mybir.dt.float32
    bf16 = mybir.dt.bfloat16
    AF = mybir.ActivationFunctionType
    ALU = mybir.AluOpType

    nc = bacc.Bacc(
        "TRN2",
        target_bir_lowering=False,
        debug=False,
        enable_asserts=False,
        num_devices=N_CORES,
    )
    xp_d = nc.dram_tensor("pred", [128, FREE], bf16, kind="ExternalInput")
    xt_d = nc.dram_tensor("targ", [128, FREE], bf16, kind="ExternalInput")
    mm_d = nc.dram_tensor("mconst", [128, 252], bf16, kind="ExternalInput")
    out_d = nc.dram_tensor("out", [128, 160], f32, kind="ExternalOutput")

    QN = FREE // 4  # 2296, multiple of WR

    with tile.TileContext(nc) as tc:
        with tc.tile_pool(name="pers", bufs=1) as pers, \
             tc.tile_pool(name="conv", bufs=1) as conv, \
             tc.tile_pool(name="band", bufs=2) as band, \
             tc.psum_pool(name="ps", bufs=1) as ps:
            accs = pers.tile([128, 160], f32, tag="accs")
            msd = pers.tile([128, 252], bf16, tag="msd")
            nc.sync.dma_start(out=msd[:, :], in_=mm_d[:, :])
            bias_t = pers.tile([128, 1], f32, tag="bias")
            nc.vector.memset(bias_t[:, :], 1e-30)

            xs = {}
            for name, dram in (("p", xp_d), ("t", xt_d)):
                x = conv.tile([128, FREE], bf16, tag=f"x_{name}")
                for qt in range(4):
                    a = qt * QN
                    nc.sync.dma_start(out=x[:, a:a + QN], in_=dram[:, a:a + QN])
                dw = conv.tile([128, FREE], bf16, tag=f"dw_{name}")
                so = conv.tile([128, FREE], bf16, tag=f"so_{name}")
                u = conv.tile([128, FREE], bf16, tag="u")
                xd = conv.tile([128, FREE], bf16, tag="xd")
                for qt in range(4):
                    a = qt * QN
                    n = QN if qt < 3 else QN - 2
                    nc.vector.tensor_sub(dw[:NP_, a:a + n],
                                         x[:NP_, a + 2:a + n + 2],
                                         x[:NP_, a:a + n])
                    nc.vector.tensor_add(u[:NP_, a:a + n],
                                         x[:NP_, a:a + n],
                                         x[:NP_, a + 2:a + n + 2])
                    nc.scalar.activation(xd[:NP_, a:a + n],
                                         x[:NP_, a + 1:a + n + 1],
                                         AF.Identity, scale=2.0)
                    nc.vector.tensor_add(so[:NP_, a:a + n],
                                         u[:NP_, a:a + n],
                                         xd[:NP_, a:a + n])
                xs[name] = (dw, so)

            def vps(t):
                """Valid (w<160) view of a [128, BF] PSUM tile."""
                return t[0:NP_, :].rearrange("p (r w) -> p r w", r=BH)[:, :, 0:160]

            def vcm(t):
                """[126, 3, 160] view of a [128, CF] compact tile."""
                return t[0:NP_, :].rearrange("p (r w) -> p r w", r=BH)


            def pieces(bi):
                """(slot offset, partition range, row range) accum pieces.

                Band 17 rows 1:3 of the hb2 block (partitions 84:126) are
                junk (h>=160); engines need 32-aligned partition bases, so
                emit correction sums C (parts 64:126, rows 1:3) and
                D (parts 64:84, rows 1:3); host takes A - C + D.
                """
                ps_ = [(0, (0, NP_), (0, BH))]
                if bi == NBAND - 1:
                    ps_.append((8, (64, NP_), (1, BH)))
                    ps_.append((16, (64, 2 * DL), (1, BH)))
                return ps_

            for bi in range(NBAND):
                j0 = BH * bi
                a = j0 * WR
                grads = {}
                for name in ("p", "t"):
                    dw, so = xs[name]
                    uh = band.tile([128, 4 * WR], bf16, tag="uh")
                    hx = band.tile([128, BF], bf16, tag="hx")
                    nc.vector.tensor_add(uh[:NP_, :],
                                         dw[:NP_, a:a + 4 * WR],
                                         dw[:NP_, a + WR:a + 5 * WR])
                    nc.vector.tensor_add(hx[:NP_, :],
                                         uh[:NP_, 0:BF],
                                         uh[:NP_, WR:WR + BF])
                    hy = band.tile([128, BF], bf16, tag="hy")
                    nc.vector.tensor_sub(hy[:NP_, :],
                                         so[:NP_, a + 2 * WR:a + 2 * WR + BF],
                                         so[:NP_, a:a + BF])
                    uh2 = band.tile([128, 4 * WR], bf16, tag="uh")
                    hz = band.tile([128, BF], bf16, tag="hz")
                    nc.vector.tensor_add(uh2[:NP_, :],
                                         so[:NP_, a:a + 4 * WR],
                                         so[:NP_, a + WR:a + 5 * WR])
                    nc.vector.tensor_add(hz[:NP_, :],
                                         uh2[:NP_, 0:BF],
                                         uh2[:NP_, WR:WR + BF])
                    gx = ps.tile([128, BF], f32, tag=f"gx_{name}")
                    gy = ps.tile([128, BF], f32, tag=f"gy_{name}")
                    gz = ps.tile([128, BF], f32, tag=f"gz_{name}")
                    nc.tensor.matmul(out=gx[0:NP_, :], lhsT=msd[0:NP_, 0:126],
                                     rhs=hx[0:NP_, :], start=True, stop=True)
                    nc.tensor.matmul(out=gy[0:NP_, :], lhsT=msd[0:NP_, 0:126],
                                     rhs=hy[0:NP_, :], start=True, stop=True)
                    nc.tensor.matmul(out=gz[0:NP_, :], lhsT=msd[0:NP_, 126:252],
                                     rhs=hz[0:NP_, :], start=True, stop=True)
                    grads[name] = (gx, gy, gz)

                # evacuate t-gradients (ScalarE copy, PSUM -> compact SBUF)
                gtc = []
                for ci in range(3):
                    g = band.tile([128, CF], bf16, tag=f"gtc{ci}")
                    nc.scalar.activation(vcm(g), vps(grads["t"][ci]),
                                         AF.Identity)
                    gtc.append(g)

                # squares (ScalarE, fused accums)
                sqs = {"p": [], "t": []}
                for ni, (name, srcs) in enumerate(
                        (("p", [vps(g) for g in grads["p"]]),
                         ("t", [vcm(g) for g in gtc]))):
                    for ci, gv in enumerate(srcs):
                        sq = band.tile([128, CF], bf16, tag=f"sq{name}{ci}")
                        sv = vcm(sq)
                        k = 3 * ni + ci
                        for off, (p0, p1), (r0, r1) in pieces(bi):
                            slot = 8 * bi + off + k
                            nc.scalar.activation(
                                sv[p0:p1, r0:r1], gv[p0:p1, r0:r1],
                                AF.Square,
                                accum_out=accs[p0:p1, slot:slot + 1])
                        sqs[name].append(sq)

                # dot products (DVE, single PSUM operand)
                ms = []
                for ci in range(3):
                    m = band.tile([128, CF], bf16, tag=f"m{ci}")
                    nc.vector.tensor_mul(vcm(m), vps(grads["p"][ci]),
                                         vcm(gtc[ci]))
                    ms.append(m)

                s_p = band.tile([128, CF], bf16, tag="s_p")
                s_t = band.tile([128, CF], bf16, tag="s_t")
                dot = band.tile([128, CF], bf16, tag="dot")
                t0 = band.tile([128, CF], bf16, tag="t0")
                nc.vector.tensor_add(t0[:NP_, :], sqs["p"][0][:NP_, :],
                                     sqs["p"][1][:NP_, :])
                nc.vector.tensor_add(s_p[:NP_, :], t0[:NP_, :],
                                     sqs["p"][2][:NP_, :])
                nc.vector.tensor_add(t0[:NP_, :], sqs["t"][0][:NP_, :],
                                     sqs["t"][1][:NP_, :])
                nc.vector.tensor_add(s_t[:NP_, :], t0[:NP_, :],
                                     sqs["t"][2][:NP_, :])
                nc.vector.tensor_add(t0[:NP_, :], ms[0][:NP_, :],
                                     ms[1][:NP_, :])
                nc.vector.tensor_add(dot[:NP_, :], t0[:NP_, :],
                                     ms[2][:NP_, :])
                q = band.tile([128, CF], bf16, tag="q")
                nc.vector.tensor_mul(q[:NP_, :], s_p[:NP_, :], s_t[:NP_, :])

                sqq = band.tile([128, CF], f32, tag="sqq")
                for off, (p0, p1), (r0, r1) in pieces(bi):
                    slot = 8 * bi + off + 6
                    nc.scalar.activation(
                        vcm(sqq)[p0:p1, r0:r1], vcm(q)[p0:p1, r0:r1],
                        AF.Sqrt, bias=bias_t[p0:p1, 0:1],
                        accum_out=accs[p0:p1, slot:slot + 1])
                r = band.tile([128, CF], f32, tag="r")
                nc.vector.reciprocal_approx_fast(out=r[:NP_, :],
                                                 in_=sqq[:NP_, :])
                cj = band.tile([128, CF], bf16, tag="cj")
                nc.vector.tensor_mul(cj[:NP_, :], dot[:NP_, :], r[:NP_, :])
                for off, (p0, p1), (r0, r1) in pieces(bi):
                    slot = 8 * bi + off + 7
                    nc.scalar.activation(
                        vcm(t0)[p0:p1, r0:r1], vcm(cj)[p0:p1, r0:r1],
                        AF.Identity,
                        accum_out=accs[p0:p1, slot:slot + 1])

            nc.sync.dma_start(out=out_d[:, :], in_=accs[:, :])

    nc.compile()
    return nc


def _shard_inputs(pred, target):
    bf = ml_dtypes.bfloat16
    in_maps = []
    padded = {}
    for name, x in (("pred", pred), ("targ", target)):
        per_b = []
        for b in range(2):
            G = np.zeros((164, 164, 164), np.float32)
            G[1:161, 1:161, 1:161] = x[b, 0]
            per_b.append(G)
        padded[name] = per_b

    for core in range(N_CORES):
        b, q = divmod(core, 4)
        m = {}
        for name in ("pred", "targ"):
            G = padded[name][b]
            slab = G[40 * q:40 * q + DL]          # [42, 164, 164]
            blocks = np.stack([slab[:, hb * 54:hb * 54 + HL, :]
                               for hb in range(HB)])  # [3, 42, 56, 164]
            arr = np.zeros((128, FREE), bf)
            arr[:NP_] = blocks.reshape(NP_, FREE).astype(bf)
            m[name] = arr
        m["mconst"] = _build_M().astype(bf)
        in_maps.append(m)
    return in_maps


def run(pred, target, trace=False):
    from concourse.bass_utils import run_bass_kernel_spmd

    pred = np.asarray(pred, dtype=np.float32)
    target = np.asarray(target, dtype=np.float32)
    assert pred.shape == (2, 1, 160, 160, 160)

    if "nc" not in _cache:
        _cache["nc"] = _build()
    nc = _cache["nc"]

    in_maps = _shard_inputs(pred, target)
    res = None
    for attempt in range(3):
        try:
            res = run_bass_kernel_spmd(
                nc, in_maps, core_ids=list(range(N_CORES)), trace=trace)
            break
        except Exception:
            if attempt == 2:
                raise
            import time as _time
            _time.sleep(5)

    sp = st = sq = cs = 0.0
    for core_out in res.results:
        o = np.asarray(core_out["out"], np.float64)
        sl = o[:NP_, :8 * NBAND].reshape(NP_, NBAND, 8)
        tk = sl.sum(axis=(0, 1))                  # A sums per quantity
        tk -= o[64:NP_, 144:152].sum(axis=0)      # - C (junk + hb1 rows)
        tk += o[64:2 * DL, 152:160].sum(axis=0)   # + D (hb1 rows back)
        sp += tk[0:3].sum()
        st += tk[3:6].sum()
        sq += tk[6]
        cs += tk[7]

    mag = sp + st - 2.0 * sq
    loss = WEIGHT * (mag / NVOX + 1.0 - cs / NVOX)
    return np.float32(loss), res.exec_time_ns


def kernel(pred, target):
    loss, _ = run(pred, target, trace=False)
    return loss


# revision 18
# speedup vs baseline: 1.2885x; 1.1441x over previous
"""Trainium2 distributed kernel for AnatomicalConsistencyLoss (v2).

Sharding: 8 cores = (batch b in {0,1}) x (depth quarter q in {0..3});
each core owns 40 output D-planes (full H,W) of one batch element.

Per-core layout: partitions p = hb*42 + dl (3 h-blocks x 42 d-planes
incl 1-plane halo), free axis = (h_local 56 incl halo, w 164 padded)
bf16.  The Sobel separable conv is split across engines:
  - W passes (stride-1 axis): VectorE shifted adds at DVE 2x bf16 mode,
    with the odd-offset center tap (2*x<<1) on ScalarE.
  - H passes: VectorE shifted adds at even 164-elem offsets (2x mode).
  - D passes: TensorE matmuls with banded [126,126] conv matrices
    (S=[1,2,1], D=[-1,0,1] per h-block, zero columns at d-halo
    outputs), streaming 3-h-row chunks into PSUM fp32.
Squares run on ScalarE straight out of PSUM (fused valid-region
compaction + accum_out partial sums); dot products are DVE muls from
PSUM; sqrt on ScalarE (+accum for the mag cross term); 1/sqrt via the
custom-DVE fast reciprocal; the cosine sum via tensor_tensor_reduce.

Per-core output: [128, 160] fp32 accum slots (8 per h-band x 18 bands:
3x sum gp^2, 3x sum gt^2, sum sqrt(q), sum dot/sqrt(q)); host reduces.
"""

import sys

import numpy as np

sys.path.insert(0, "/opt/trn_rl_repo")

import ml_dtypes

N_CORES = 8
DL = 42            # d planes incl halo
HB = 3             # h blocks
HL = 56            # h_local rows incl halo
WR = 164           # padded w row (4B-aligned rows)
NP_ = 126          # used partitions
FREE = HL * WR     # 9184
NBAND = 9          # 54 valid h rows / 6
BH = 6             # h rows per band
BF = 3 * WR        # 492 cols per matmul chunk (<= 512 fp32 bank)
PF = 1024          # PSUM tile cols (2 banks; rows 0-2 at 0, 3-5 at 512)
CF = BH * 160      # 960 compact cols
NVOX = 2 * 160 * 160 * 160
WEIGHT = 0.2

_cache = {}


def _build_M():
    MS = np.zeros((128, 252), np.float32)
    for hb in range(HB):
        for do in range(40):
            j = hb * DL + do
            MS[hb * DL + do, j] += 1.0
            MS[hb * DL + do + 1, j] += 2.0
            MS[hb * DL + do + 2, j] += 1.0
            MS[hb * DL + do, 126 + j] += -1.0
            MS[hb * DL + do + 2, 126 + j] += 1.0
    return MS


def _build():
    import concourse.bacc as bacc
    import concourse.tile as tile
    from concourse import mybir

    f32 = mybir.dt.float32
    bf16 = mybir.dt.bfloat16
    AF = mybir.ActivationFunctionType
    ALU = mybir.AluOpType

    nc = bacc.Bacc(
        "TRN2",
        target_bir_lowering=False,
        debug=False,
        enable_asserts=False,
        num_devices=N_CORES,
    )
    xp_d = nc.dram_tensor("pred", [128, FREE], bf16, kind="ExternalInput")
    xt_d = nc.dram_tensor("targ", [128, FREE], bf16, kind="ExternalInput")
    mm_d = nc.dram_tensor("mconst", [128, 252], bf16, kind="ExternalInput")
    out_d = nc.dram_tensor("out", [128, 160], f32, kind="ExternalOutput")

    QN = FREE // 4  # 2296, multiple of WR

    with tile.TileContext(nc) as tc:
        with tc.tile_pool(name="pers", bufs=1) as pers, \
             tc.tile_pool(name="conv", bufs=1) as conv, \
             tc.tile_pool(name="band", bufs=2) as band, \
             tc.psum_pool(name="ps", bufs=1) as ps:
            accs = pers.tile([128, 160], f32, tag="accs")
            msd = pers.tile([128, 252], bf16, tag="msd")
            nc.sync.dma_start(out=msd[:, :], in_=mm_d[:, :])
            bias_t = pers.tile([128, 1], f32, tag="bias")
            nc.vector.memset(bias_t[:, :], 1e-30)

            xs = {}
            for name, dram in (("p", xp_d), ("t", xt_d)):
                x = conv.tile([128, FREE], bf16, tag=f"x_{name}")
                for qt in range(4):
                    a = qt * QN
                    nc.sync.dma_start(out=x[:, a:a + QN], in_=dram[:, a:a + QN])
                dw = conv.tile([128, FREE], bf16, tag=f"dw_{name}")
                so = conv.tile([128, FREE], bf16, tag=f"so_{name}")
                for qt in range(4):
                    a = qt * QN
                    n = QN if qt < 3 else QN - 2
                    u = conv.tile([128, QN], bf16, tag="u")
                    xd = conv.tile([128, QN], bf16, tag="xd")
                    nc.vector.tensor_sub(dw[:NP_, a:a + n],
                                         x[:NP_, a + 2:a + n + 2],
                                         x[:NP_, a:a + n])
                    nc.vector.tensor_add(u[:NP_, 0:n],
                                         x[:NP_, a:a + n],
                                         x[:NP_, a + 2:a + n + 2])
                    nc.scalar.activation(xd[:NP_, 0:n],
                                         x[:NP_, a + 1:a + n + 1],
                                         AF.Identity, scale=2.0)
                    nc.vector.tensor_add(so[:NP_, a:a + n],
                                         u[:NP_, 0:n],
                                         xd[:NP_, 0:n])
                xs[name] = (dw, so)

            def vps(t):
                """Valid [126, 2, 3, 160] view of a [128, PF] PSUM tile.

                Row r (0..5) lives at col 512*(r//3) + 164*(r%3) so each
                3-row chunk stays inside one 512-fp32 PSUM bank.
                """
                return (t[0:NP_, :]
                        .rearrange("p (c q) -> p c q", c=2)[:, :, 0:BF]
                        .rearrange("p c (k w) -> p c k w", k=3)[:, :, :, 0:160])

            def vcm(t):
                """[126, 2, 3, 160] view of a [128, CF] compact tile."""
                return t[0:NP_, :].rearrange("p (c k w) -> p c k w", c=2, k=3)

            def pieces(bi):
                """(slot offset, partition range, (c, k) row slices).

                Last band rows 4:6 of the hb2 block (partitions 84:126) are
                junk (h >= 160); engines need 32-aligned partition bases, so
                emit correction sums C (parts 64:126, rows 4:6) and
                D (parts 64:84, rows 4:6); host takes A - C + D.
                """
                ps_ = [(0, (0, NP_), (0, 2), (0, 3))]
                if bi == NBAND - 1:
                    ps_.append((8, (64, NP_), (1, 2), (1, 3)))
                    ps_.append((16, (64, 2 * DL), (1, 2), (1, 3)))
                return ps_

            for bi in range(NBAND):
                a = BH * bi * WR
                gtc = []
                grads_p = []
                gps = [ps.tile([128, PF], f32, tag=f"g{ci}", name=f"g{ci}")
                       for ci in range(3)]
                for name in ("t", "p"):
                    dw, so = xs[name]
                    uh = band.tile([128, 7 * WR], bf16, tag="uh")
                    hx = band.tile([128, BH * WR], bf16, tag="hx")
                    nc.vector.tensor_add(uh[:NP_, :],
                                         dw[:NP_, a:a + 7 * WR],
                                         dw[:NP_, a + WR:a + 8 * WR])
                    nc.vector.tensor_add(hx[:NP_, :],
                                         uh[:NP_, 0:BH * WR],
                                         uh[:NP_, WR:WR + BH * WR])
                    hy = band.tile([128, BH * WR], bf16, tag="hy")
                    nc.vector.tensor_sub(hy[:NP_, :],
                                         so[:NP_, a + 2 * WR:a + (2 + BH) * WR],
                                         so[:NP_, a:a + BH * WR])
                    uh2 = band.tile([128, 7 * WR], bf16, tag="uh")
                    hz = band.tile([128, BH * WR], bf16, tag="hz")
                    nc.vector.tensor_add(uh2[:NP_, :],
                                         so[:NP_, a:a + 7 * WR],
                                         so[:NP_, a + WR:a + 8 * WR])
                    nc.vector.tensor_add(hz[:NP_, :],
                                         uh2[:NP_, 0:BH * WR],
                                         uh2[:NP_, WR:WR + BH * WR])
                    for ci, (rhs, wcol) in enumerate(
                            ((hx, 0), (hy, 0), (hz, 126))):
                        for ch in range(2):
                            nc.tensor.matmul(
                                out=gps[ci][0:NP_, 512 * ch:512 * ch + BF],
                                lhsT=msd[0:NP_, wcol:wcol + 126],
                                rhs=rhs[0:NP_, BF * ch:BF * (ch + 1)],
                                start=True, stop=True)
                    if name == "t":
                        # evacuate t-gradients so p can reuse the banks
                        for ci in range(3):
                            g = band.tile([128, CF], bf16, tag=f"gtc{ci}")
                            nc.scalar.activation(vcm(g), vps(gps[ci]),
                                                 AF.Identity)
                            gtc.append(g)
                    else:
                        grads_p = gps

                # squares (ScalarE, fused accums)
                sqs = {"p": [], "t": []}
                for ni, (name, srcs) in enumerate(
                        (("p", [vps(g) for g in grads_p]),
                         ("t", [vcm(g) for g in gtc]))):
                    for ci, gv in enumerate(srcs):
                        sq = band.tile([128, CF], bf16, tag=f"sq{name}{ci}")
                        sv = vcm(sq)
                        k = 3 * ni + ci
                        for off, (p0, p1), cc, kk in pieces(bi):
                            slot = 8 * bi + off + k
                            nc.scalar.activation(
                                sv[p0:p1, cc[0]:cc[1], kk[0]:kk[1]],
                                gv[p0:p1, cc[0]:cc[1], kk[0]:kk[1]],
                                AF.Square,
                                accum_out=accs[p0:p1, slot:slot + 1])
                        sqs[name].append(sq)

                s_p = band.tile([128, CF], bf16, tag="s_p")
                s_t = band.tile([128, CF], bf16, tag="s_t")
                dot = band.tile([128, CF], bf16, tag="dot")
                t0 = band.tile([128, CF], bf16, tag="t0")
                nc.vector.tensor_add(t0[:NP_, :], sqs["p"][0][:NP_, :],
                                     sqs["p"][1][:NP_, :])
                nc.vector.tensor_add(s_p[:NP_, :], t0[:NP_, :],
                                     sqs["p"][2][:NP_, :])
                nc.vector.tensor_add(t0[:NP_, :], sqs["t"][0][:NP_, :],
                                     sqs["t"][1][:NP_, :])
                nc.vector.tensor_add(s_t[:NP_, :], t0[:NP_, :],
                                     sqs["t"][2][:NP_, :])

                # dot products (DVE, single PSUM operand); reuse sqp memory
                ms = []
                for ci in range(3):
                    m = band.tile([128, CF], bf16, tag=f"sqp{ci}", name=f"m{ci}")
                    nc.vector.tensor_mul(vcm(m), vps(grads_p[ci]),
                                         vcm(gtc[ci]))
                    ms.append(m)
                nc.vector.tensor_add(t0[:NP_, :], ms[0][:NP_, :],
                                     ms[1][:NP_, :])
                nc.vector.tensor_add(dot[:NP_, :], t0[:NP_, :],
                                     ms[2][:NP_, :])
                q = band.tile([128, CF], bf16, tag="q")
                nc.vector.tensor_mul(q[:NP_, :], s_p[:NP_, :], s_t[:NP_, :])

                sqq = band.tile([128, CF], f32, tag="sqq")
                for off, (p0, p1), cc, kk in pieces(bi):
                    slot = 8 * bi + off + 6
                    nc.scalar.activation(
                        vcm(sqq)[p0:p1, cc[0]:cc[1], kk[0]:kk[1]],
                        vcm(q)[p0:p1, cc[0]:cc[1], kk[0]:kk[1]],
                        AF.Sqrt, bias=bias_t[p0:p1, 0:1],
                        accum_out=accs[p0:p1, slot:slot + 1])
                r = band.tile([128, CF], f32, tag="r")
                nc.vector.reciprocal_approx_fast(out=r[:NP_, :],
                                                 in_=sqq[:NP_, :])
                cj = band.tile([128, CF], bf16, tag="cj")
                nc.vector.tensor_mul(cj[:NP_, :], dot[:NP_, :], r[:NP_, :])
                for off, (p0, p1), cc, kk in pieces(bi):
                    slot = 8 * bi + off + 7
                    nc.scalar.activation(
                        vcm(t0)[p0:p1, cc[0]:cc[1], kk[0]:kk[1]],
                        vcm(cj)[p0:p1, cc[0]:cc[1], kk[0]:kk[1]],
                        AF.Identity,
                        accum_out=accs[p0:p1, slot:slot + 1])

            nc.sync.dma_start(out=out_d[:, :], in_=accs[:, :])

    nc.compile()
    return nc


def _shard_inputs(pred, target):
    bf = ml_dtypes.bfloat16
    in_maps = []
    padded = {}
    for name, x in (("pred", pred), ("targ", target)):
        per_b = []
        for b in range(2):
            G = np.zeros((164, 164, 164), np.float32)
            G[1:161, 1:161, 1:161] = x[b, 0]
            per_b.append(G)
        padded[name] = per_b

    for core in range(N_CORES):
        b, q = divmod(core, 4)
        m = {}
        for name in ("pred", "targ"):
            G = padded[name][b]
            slab = G[40 * q:40 * q + DL]          # [42, 164, 164]
            blocks = np.stack([slab[:, hb * 54:hb * 54 + HL, :]
                               for hb in range(HB)])  # [3, 42, 56, 164]
            arr = np.zeros((128, FREE), bf)
            arr[:NP_] = blocks.reshape(NP_, FREE).astype(bf)
            m[name] = arr
        m["mconst"] = _build_M().astype(bf)
        in_maps.append(m)
    return in_maps


def run(pred, target, trace=False):
    from concourse.bass_utils import run_bass_kernel_spmd

    pred = np.asarray(pred, dtype=np.float32)
    target = np.asarray(target, dtype=np.float32)
    assert pred.shape == (2, 1, 160, 160, 160)

    if "nc" not in _cache:
        _cache["nc"] = _build()
    nc = _cache["nc"]

    in_maps = _shard_inputs(pred, target)
    res = None
    for attempt in range(3):
        try:
            res = run_bass_kernel_spmd(
                nc, in_maps, core_ids=list(range(N_CORES)), trace=trace)
            break
        except Exception:
            if attempt == 2:
                raise
            import time as _time
            _time.sleep(5)

    sp = st = sq = cs = 0.0
    nb8 = 8 * NBAND
    for core_out in res.results:
        o = np.asarray(core_out["out"], np.float64)
        sl = o[:NP_, :nb8].reshape(NP_, NBAND, 8)
        tk = sl.sum(axis=(0, 1))                       # A sums per quantity
        tk -= o[64:NP_, nb8 + 8:nb8 + 16].sum(axis=0)  # - C (junk + hb1)
        tk += o[64:2 * DL, nb8 + 16:nb8 + 24].sum(axis=0)  # + D (hb1 back)
        sp += tk[0:3].sum()
        st += tk[3:6].sum()
        sq += tk[6]
        cs += tk[7]

    mag = sp + st - 2.0 * sq
    loss = WEIGHT * (mag / NVOX + 1.0 - cs / NVOX)
    return np.float32(loss), res.exec_time_ns


def kernel(pred, target):
    loss, _ = run(pred, target, trace=False)
    return loss
